# revision 8
# baseline (speedup 1.0000x reference)
"""Trainium2 Bass kernel for the AllGroupsExpertRunner MoE problem.

Math (dense-masked reference):
    x = tokens.reshape(M, D)                                # M = B*N = 8192
    out = sum_e w[:, e] * (gelu(x @ Wg[e]) * (x @ Wv[e])) @ Wo[e] * scales[e]
    where w = where(dispatch > 0, combine, 0)

Tokens with w[:, e] == 0 contribute nothing for expert e, so each expert only
needs its assigned tokens. Sharding: expert-parallel — core e runs expert e on
the tokens routed to it (gathered on host, padded to a common length NT), and
the host scatter-adds the 8 partial outputs. With top-2 routing that is ~2100
of 8192 tokens per expert (~4x less work than dense); with dense routing
weights it degrades gracefully to all tokens.

Per-core kernel: all matmuls run on the PE array in float32r (fp32 data at
full 1 cycle/row rate; ~2.8e-4 rel err measured on HW). x is passed
pre-transposed (D, NT) so no on-device transposes are needed:
  stage A (per token chunk of <=512, per 128-wide H block):
      g^T = Wg_blk^T @ xT-chunk  (PSUM, K=D via 4 accumulating matmuls)
      v^T = Wv_blk^T @ xT-chunk
      hT_blk = gelu(g^T) * v^T   (ACT + DVE)
  stage B (per 128-token sub-chunk):
      out = hT^T @ Wo (16 accumulating matmuls over H), scaled per-token by
      the routing weight (DVE per-partition scalar), DMA'd out.
Weights are loaded as 16 tiles of (128, 512) per tensor so the first matmuls
only wait on the first 256KB of DMA instead of the full 12.6MB.
"""

import numpy as np

D = 512
H = 2048
E = 8
P = 128
MT = 512  # max token chunk (fp32 moving-operand limit)
ND = D // P  # 4 k-tiles over D
NH = H // P  # 16 k-tiles over H
NJ = 4  # column chunks per weight d-tile (H / 512)

_CACHE: dict = {}


def _build_program(NT: int):
    from contextlib import ExitStack

    import concourse.bacc as bacc
    import concourse.tile as tile
    import concourse.mybir as mybir

    assert NT % P == 0
    f32 = mybir.dt.float32
    DT = mybir.dt.float32r

    nc = bacc.Bacc("TRN2", target_bir_lowering=False, debug=False)

    xT = nc.dram_tensor("xT", [D, NT], DT, kind="ExternalInput")
    wg = nc.dram_tensor("wg", [D, H], DT, kind="ExternalInput")
    wv = nc.dram_tensor("wv", [D, H], DT, kind="ExternalInput")
    wo = nc.dram_tensor("wo", [H, D], DT, kind="ExternalInput")
    wc = nc.dram_tensor("wc", [P, NT // P], f32, kind="ExternalInput")
    out = nc.dram_tensor("out", [NT, D], f32, kind="ExternalOutput")

    chunks = [MT] * (NT // MT)
    if NT % MT:
        chunks.append(NT % MT)
    gelu = mybir.ActivationFunctionType.Gelu

    with tile.TileContext(nc) as tc, ExitStack() as ctx:
        wpool = ctx.enter_context(tc.tile_pool(name="w", bufs=1))
        xpool = ctx.enter_context(tc.tile_pool(name="x", bufs=3))
        hpool = ctx.enter_context(tc.tile_pool(name="h", bufs=1))
        gpool = ctx.enter_context(tc.tile_pool(name="g", bufs=3))
        opool = ctx.enter_context(tc.tile_pool(name="o", bufs=4))
        psg = ctx.enter_context(tc.tile_pool(name="psg", bufs=2, space="PSUM"))
        psv = ctx.enter_context(tc.tile_pool(name="psv", bufs=2, space="PSUM"))
        pso = ctx.enter_context(tc.tile_pool(name="pso", bufs=2, space="PSUM"))

        # weight tiles, (128, 512) each, DMA'd in first-use order.
        # Everything on one HWDGE queue serializes (~52us before the first
        # matmul), so: chunk-0 tokens go first on the SP queue, Wg streams on
        # the ACT queue, Wv on the SP queue, Wo on the GpSimd SWDGE queues.
        wg_t = [[wpool.tile([P, MT], DT, tag=f"wg{d}_{j}", name=f"wg{d}_{j}") for j in range(NJ)]
                for d in range(ND)]
        wv_t = [[wpool.tile([P, MT], DT, tag=f"wv{d}_{j}", name=f"wv{d}_{j}") for j in range(NJ)]
                for d in range(ND)]
        wo_t = [wpool.tile([P, D], DT, tag=f"wo{h}", name=f"wo{h}") for h in range(NH)]
        wc_t = wpool.tile([P, NT // P], f32, tag="wc")

        mt0 = chunks[0]
        xq0 = [xpool.tile([P, mt0], DT, tag=f"xq{d}", name=f"xq{d}") for d in range(ND)]
        for d in range(ND):
            nc.sync.dma_start(out=xq0[d][:], in_=xT[d * P : (d + 1) * P, 0:mt0])
        nc.sync.dma_start(out=wc_t[:], in_=wc[:])
        # arrival matched to demand on two in-order HWDGE queues: ACT carries
        # wg then wo, SP carries wv then per-chunk xq; wo tail on SWDGE
        for j in range(NJ):
            for d in range(ND):
                nc.scalar.dma_start(
                    out=wg_t[d][j][:],
                    in_=wg[d * P : (d + 1) * P, j * MT : (j + 1) * MT],
                )
            for d in range(ND):
                nc.sync.dma_start(
                    out=wv_t[d][j][:],
                    in_=wv[d * P : (d + 1) * P, j * MT : (j + 1) * MT],
                )
        for h in range(NH - 4):
            nc.scalar.dma_start(out=wo_t[h][:], in_=wo[h * P : (h + 1) * P, :])
        for h in range(NH - 4, NH):
            nc.gpsimd.dma_start(out=wo_t[h][:], in_=wo[h * P : (h + 1) * P, :])

        tok0 = 0
        for ci, mt in enumerate(chunks):
            if ci == 0:
                xq = xq0
            else:
                xq = [xpool.tile([P, mt], DT, tag=f"xq{d}", name=f"xq{d}") for d in range(ND)]
                for d in range(ND):
                    nc.sync.dma_start(
                        out=xq[d][:], in_=xT[d * P : (d + 1) * P, tok0 : tok0 + mt]
                    )
            hT = hpool.tile([P, NH, mt], DT, tag="hT")
            for h in range(NH):
                j, hc = divmod(h, NJ)
                hs = slice(hc * P, (hc + 1) * P)
                pg = psg.tile([P, mt], f32, tag="pg")
                pv = psv.tile([P, mt], f32, tag="pv")
                for d in range(ND):
                    nc.tensor.matmul(
                        out=pg[:], lhsT=wg_t[d][j][:, hs], rhs=xq[d][:],
                        start=(d == 0), stop=(d == ND - 1),
                    )
                for d in range(ND):
                    nc.tensor.matmul(
                        out=pv[:], lhsT=wv_t[d][j][:, hs], rhs=xq[d][:],
                        start=(d == 0), stop=(d == ND - 1),
                    )
                ga = gpool.tile([P, mt], DT, tag="ga")
                nc.scalar.activation(ga[:], pg[:], gelu)
                nc.vector.tensor_mul(hT[:, h, :], ga[:], pv[:])
            for t in range(mt // P):
                po = pso.tile([P, D], f32, tag="po")
                for h in range(NH):
                    nc.tensor.matmul(
                        out=po[:], lhsT=hT[:, h, t * P : (t + 1) * P],
                        rhs=wo_t[h][:], start=(h == 0), stop=(h == NH - 1),
                    )
                ob = opool.tile([P, D], f32, tag="ob")
                j = tok0 // P + t
                nc.vector.tensor_scalar_mul(ob[:], po[:], wc_t[:, j : j + 1])
                nc.scalar.dma_start(out=out[j * P : (j + 1) * P, :], in_=ob[:])
            tok0 += mt

    nc.compile()
    return nc


def kernel(tokens, dispatch_weights, combine_weights, Wg, Wv, Wo, scales):
    from concourse.bass_utils import run_bass_kernel_spmd

    B, N, d_model = tokens.shape
    M = B * N
    x = np.ascontiguousarray(tokens.reshape(M, d_model), dtype=np.float32)
    disp = np.asarray(dispatch_weights).reshape(M, E)
    comb = np.asarray(combine_weights).reshape(M, E)
    w_all = np.where(disp > 0, comb, 0.0).astype(np.float32) * np.asarray(
        scales, np.float32
    )[None, :]

    idx = [np.nonzero(w_all[:, e])[0] for e in range(E)]
    n_max = max(1, max(len(i) for i in idx))
    # pad to a multiple of 256: float32r matmuls need a moving dim >= 256 to
    # run at full rate, so a 128-wide tail chunk would cost as much as 512
    NT = ((n_max + 255) // 256) * 256

    if NT not in _CACHE:
        _CACHE[NT] = _build_program(NT)
    nc = _CACHE[NT]

    in_maps = []
    for e in range(E):
        ie = idx[e]
        ne = len(ie)
        xT_e = np.zeros((D, NT), np.float32)
        xT_e[:, :ne] = x[ie].T
        wc_e = np.zeros((NT // P, P), np.float32)
        wc_e.reshape(-1)[:ne] = w_all[ie, e]
        in_maps.append(
            {
                "xT": xT_e,
                "wg": np.ascontiguousarray(Wg[e], np.float32),
                "wv": np.ascontiguousarray(Wv[e], np.float32),
                "wo": np.ascontiguousarray(Wo[e], np.float32),
                "wc": np.ascontiguousarray(wc_e.T),
            }
        )

    res = run_bass_kernel_spmd(nc, in_maps, list(range(E)))

    out = np.zeros((M, d_model), np.float32)
    for e in range(E):
        ne = len(idx[e])
        out[idx[e]] += res.results[e]["out"][:ne]
    return out.reshape(B, N, d_model)
